# revision 24
# baseline (speedup 1.0000x reference)
"""Expert-parallel grouped matmul (MoE BatchLinear) for 8 Trainium2 NeuronCores.

Problem: y[t] = x[t] @ W[g(t)] where tokens are grouped contiguously by expert
g (G=64 experts, counts given at runtime). Sharding: expert-parallel — core c
owns experts [8c, 8c+8) and the contiguous token rows routed to them. The
"all-to-all" is done host-side: kernel() receives full inputs, slices/pads
per-core token blocks, and scatters per-core outputs back.

Device kernel (SPMD, one program on 8 cores):
  ~27 warmup matmuls on a memset tile bridge the first-DMA wait so the PE
  DVFS clock is at 2.4 GHz when real data lands (cold-start matmuls
  otherwise run at 1.2 GHz for ~3 us).
  lead expert (big, 6 m-tiles): the first xt m-tiles and W chunk are
    k-split into half-tiles across both HWDGE rings so the first matmul is
    gated on 128 KB, and the (chunk, m) pairs are processed in a wavefront
    order matched to the staged DMA arrivals (m1, c1, m2, c2, c3, m3..)
    so the hot PE never runs out of arrived work. The narrow-chunk prefix
    (cols 0..1024) casts into one staging tile per m with a single y write
    (42 individual y triggers at ~0.7 us each would outrun the scalar
    sequencer and back up PSUM).
  remaining experts (big/small interleaved): chunk-major 1024-wide W
    chunks; per (chunk, m): 8 k-steps x nb matmuls (N<=512) accumulate in
    PSUM, DVE cast PSUM -> SBUF fp16 staging, ACT-ring DMA staging -> y.

Key perf choices (each traced on HW):
  - W + non-lead xT on the SP ring; y alone on the ACT ring. A pure-y
    scalar queue means the cast st-slot gates (which count scalar-queue
    completion sems) can never be held hostage by an xt trigger waiting on
    far-future PE progress -- that entanglement was a ~6 us/pair crawl
    (PE -> cast -> y-queue -> xt-trigger -> PE) costing 5-7 us on cores
    whose DMA timing landed badly.
  - xt bufs=13: xt-slot recycling stays ~2 experts behind the allocation,
    so xt triggers never carry a PE-progress wait at all.
  - 1024-wide W chunks, 10-buf pool: chunk triggers never wait on slot
    recycling; the interleaved big/small expert order builds W-prefetch
    credit during big experts for the 295 GB/s small-expert burn.
  - y as fp16 (host upcasts): halves y HBM traffic; keeps rings unsaturated.
  - ladder widths [128,128,256,512,1024..]: the 128-wide chunks run at half
    PE rate (LDWEIGHTS-limited, 97 ns vs 53 ns stream) but live entirely
    inside the DMA-bound cold-start window, so the narrowness is free --
    measured: widening them to 384/512 costs 7 us of arrival stalls.

Numerics: operands stream as fp16 (1 PE cycle/row, fp32 PSUM accumulation);
y returns as fp16 and is upcast host-side. Measured: absmax/scale 4.8e-4,
rms rel 3.6e-4 (gates: 2e-3 / 2e-2). fp8 double-pumping (2x PE) was
analyzed and rejected: e4m3 quantization alone is ~2.7e-2 rms / ~3e-2
absmax, over both gates, and any hi+lo compensation needs >=2 fp8 MACs per
logical MAC = no speedup over fp16.

Perf (HW NTFF, all 8 cores): 461.5-464.6 us across cores (was 461-470 with
the crawl; staged baseline 467.8). Breakdown vs the 442.4 us all-N=512
stream floor (2048 matmuls x 216 ns at 2.4 GHz incl. 2.2 ns/inst hwdecode):
preamble 6.5 + first-data 3.8 (DMA desc fetch ~1.5 + 128 KB cold transfer
+ completion-sem ~1.3) + early arrival gaps 2-4 + drain tail ~4.9 (last
cast 0.4 + trigger 0.6 + desc 1.5 + sem/drain + exit barriers ~1.7).
Occasional +8-15 us when the chip drops to a lower P-state (device-state).
"""

import numpy as np

G, N_TOK, D_IN, D_OUT, CAP = 64, 32768, 1024, 4096, 768
M_CORES = 8
EPC = G // M_CORES          # experts per core
P = 128                     # partitions / k-tile / m-tile
KO = D_IN // P              # 8 k-tiles
ORDER = "bigfirst"          # "bigfirst" | "natural"
N_WARMUP = 27               # PE-clock warmup matmuls (N=128 each)
N_XSPLIT = 3                # lead-expert xt m-tiles loaded as k-halves


def _lead_pair_order(n_chunks, mtx):
    """(chunk, m) processing order for the lead expert: a wavefront matched
    to the staged DMA arrival order (chunks and m-tiles interleave across
    both rings), so the hot PE always has runnable pairs instead of
    stalling chunk-major on the not-yet-arrived m-tiles."""
    if mtx <= 1 or n_chunks <= 1:
        return [(c, m) for c in range(n_chunks) for m in range(mtx)]
    ev = [("m", 1), ("c", 1), ("m", 2), ("c", 2), ("c", 3)]
    ev += [("m", i) for i in range(3, mtx)]
    ev += [("c", i) for i in range(4, n_chunks)]
    ev = [
        (t, i)
        for t, i in ev
        if (t == "m" and i < mtx) or (t == "c" and i < n_chunks)
    ]
    pairs = [(0, 0)]
    ac, am = 1, 1
    for t, i in ev:
        if t == "m":
            pairs += [(c, i) for c in range(ac)]
            am = max(am, i + 1)
        else:
            pairs += [(i, m) for m in range(am)]
            ac = max(ac, i + 1)
    assert sorted(pairs) == [(c, m) for c in range(n_chunks) for m in range(mtx)]
    return pairs

_cache = {}


def _slot_order(mt):
    """Expert processing order. bigfirst interleaves big/small (768,256,...)
    so the first expert's long compute builds W-prefetch credit."""
    alive = [j for j in range(EPC) if mt[j] > 0]
    if ORDER == "natural":
        return alive
    big = sorted(alive, key=lambda s: -mt[s])
    bigs = big[: (len(alive) + 1) // 2]
    small = [s for s in alive if s not in bigs]
    # stable interleave: biggest, smallest-partner, next-biggest, ...
    out = []
    for a, b in zip(bigs, small + [None]):
        out.append(a)
        if b is not None:
            out.append(b)
    if len(bigs) > len(small) + 1:
        out += bigs[len(small) + 1:]
    return out


def _widths(ei, n):
    """W chunk widths per expert: ascending ladder on the first expert (small
    time-to-first-matmul), descending on the last (small drain tail)."""
    lead, tail = ei == 0, ei == n - 1
    if lead and tail:
        return [128, 128, 256, 512, 1024, 1024, 512, 256, 128, 128]
    if lead:
        return [128, 128, 256, 512, 1024, 1024, 1024]
    if tail:
        return [1024, 1024, 1024, 512, 256, 256]
    return [1024, 1024, 1024, 1024]


def _build(mt):
    """Compile the SPMD program for per-expert-slot m-tile counts mt (len EPC)."""
    import concourse.mybir as mybir
    import concourse.tile as tile
    from concourse import bacc

    f32 = mybir.dt.float32
    f16 = mybir.dt.float16
    n_mtiles = sum(mt)
    order = _slot_order(mt)
    ne = len(order)

    nc = bacc.Bacc("TRN2", target_bir_lowering=False, debug=False)
    xt_d = {
        e: nc.dram_tensor(f"xT{e}", [mt[e], P, KO, P], f16, kind="ExternalInput")
        for e in order
    }
    w_d = {
        (e, ci): nc.dram_tensor(f"W{e}_{ci}", [P, KO, wd], f16, kind="ExternalInput")
        for ei, e in enumerate(order)
        for ci, wd in enumerate(_widths(ei, ne))
    }
    y_d = nc.dram_tensor("y", [n_mtiles, P, D_OUT], f16, kind="ExternalOutput")
    y = y_d.ap()

    KH = KO // 2  # k-split point for the startup half-tiles
    with tile.TileContext(nc) as tc:
        with (
            tc.tile_pool(name="wq", bufs=10) as wq_pool,
            tc.tile_pool(name="xt", bufs=13) as xt_pool,
            tc.tile_pool(name="st", bufs=10) as st_pool,
            tc.tile_pool(name="ps", bufs=4, space="PSUM") as ps_pool,
            tc.tile_pool(name="wu", bufs=1) as wu_pool,
        ):
            # --- PE clock warmup: the DVFS ramp needs ~3us of continuous PE
            # activity; run it during the first-DMA wait so the real matmul
            # stream starts at full clock. Operands are a tiny memset tile;
            # the PSUM result is never read (dead write, recycled later).
            wux = wu_pool.tile([P, P], f16, tag="wu", name="wux")
            nc.gpsimd.memset(wux[:], 1.0)
            wups = ps_pool.tile([P, P], f32, tag="ps", name="wups")
            for _ in range(N_WARMUP):
                nc.tensor.matmul(wups[:], wux[:], wux[:], start=True, stop=True)

            mi0 = 0  # global m-tile index
            wqs = {}
            for ei, e in enumerate(order):
                # xts[m] is either a whole [P, KO, P] tile or, for the first
                # few m-tiles of the kernel, a (lo, hi) pair of [P, KH, P]
                # half-tiles so early matmuls are gated on half the bytes.
                xts = []
                for m in range(mt[e]):
                    if ei == 0 and m < N_XSPLIT:
                        xa_ = xt_pool.tile([P, KH, P], f16, tag="xt", name="xt")
                        xb_ = xt_pool.tile([P, KH, P], f16, tag="xt", name="xt")
                        xts.append((xa_, xb_))
                    else:
                        xts.append(xt_pool.tile([P, KO, P], f16, tag="xt", name="xt"))

                def _lhsT(m, k):
                    x = xts[m]
                    if isinstance(x, tuple):
                        return x[0][:, k, :] if k < KH else x[1][:, k - KH, :]
                    return x[:, k, :]

                def _load_xt(m):
                    nc.scalar.dma_start(out=xts[m][:], in_=xt_d[e].ap()[m])

                widths = _widths(ei, ne)
                cols = [0]
                for wd in widths:
                    cols.append(cols[-1] + wd)

                # narrow-chunk prefix of the lead-expert ladder: casts gather
                # into one [P, a_cols] staging tile per m and write y once,
                # so the wavefront's many small pairs don't flood the scalar
                # ring with y-DMA triggers (trigger cost ~0.7us each).
                a_last = -1
                if ei == 0:
                    while a_last + 1 < len(widths) and cols[a_last + 2] <= 1024:
                        a_last += 1
                a_cols = cols[a_last + 1]
                st_a = {}

                def _pair(ci, m, wt_of):
                    """k-loop + cast + y write for one (chunk, m) pair."""
                    wd = widths[ci]
                    nb = (wd + 511) // 512
                    # one (up to 2-bank) PSUM tile per pair; each matmul
                    # still writes within a single bank, and the whole
                    # group evacuates in one DVE cast
                    ps = ps_pool.tile([P, wd], f32, tag="ps", name="ps")
                    for k in range(KO):
                        lhsT = _lhsT(m, k)
                        wt, wk = wt_of(ci, k)
                        for nn in range(nb):
                            w_nn = min(512, wd - nn * 512)
                            nc.tensor.matmul(
                                ps[:, nn * 512 : nn * 512 + w_nn],
                                lhsT,
                                wt[:, wk, nn * 512 : nn * 512 + w_nn],
                                start=(k == 0),
                                stop=(k == KO - 1),
                            )
                    if ci <= a_last:
                        if m not in st_a:
                            st_a[m] = st_pool.tile(
                                [P, a_cols], f16, tag="st", name="st"
                            )
                        nc.vector.tensor_copy(
                            st_a[m][:, cols[ci] : cols[ci] + wd], ps[:]
                        )
                        if ci == a_last:
                            nc.scalar.dma_start(
                                out=y[mi0 + m, :, 0:a_cols], in_=st_a.pop(m)[:]
                            )
                        return
                    st = st_pool.tile([P, wd], f16, tag="st", name="st")
                    nc.vector.tensor_copy(st[:], ps[:])
                    nc.scalar.dma_start(
                        out=y[mi0 + m, :, cols[ci] : cols[ci] + wd], in_=st[:]
                    )

                if ei == 0:
                    # Startup: the first matmul is gated only by the k<KH
                    # halves of xt m0 (scalar ring) + W chunk 0 (sync ring).
                    # Loads are staged across both rings in the same order
                    # the wavefront consumes them.
                    xa, xb = xts[0]
                    wd0 = widths[0]
                    wa0 = wq_pool.tile([P, KH, wd0], f16, tag="wq", name="wq")
                    wb0 = wq_pool.tile([P, KH, wd0], f16, tag="wq", name="wq")
                    wt = {}
                    for ci in range(1, ne_ch := len(widths)):
                        wt[ci] = wq_pool.tile(
                            [P, KO, widths[ci]], f16, tag="wq", name="wq"
                        )
                    def _load_any(m, eng):
                        x = xts[m]
                        if isinstance(x, tuple):
                            eng.dma_start(out=x[0][:], in_=xt_d[e].ap()[m][:, :KH, :])
                            eng.dma_start(out=x[1][:], in_=xt_d[e].ap()[m][:, KH:, :])
                        else:
                            eng.dma_start(out=x[:], in_=xt_d[e].ap()[m])

                    nc.scalar.dma_start(out=xa[:], in_=xt_d[e].ap()[0][:, :KH, :])
                    nc.sync.dma_start(out=wa0[:], in_=w_d[(e, 0)].ap()[:, :KH, :])
                    nc.scalar.dma_start(out=xb[:], in_=xt_d[e].ap()[0][:, KH:, :])
                    nc.sync.dma_start(out=wb0[:], in_=w_d[(e, 0)].ap()[:, KH:, :])
                    if ne_ch > 1:
                        nc.scalar.dma_start(out=wt[1][:], in_=w_d[(e, 1)].ap())
                    if mt[e] > 1:
                        _load_any(1, nc.sync)
                    for ci in (2, 3):
                        if ci < ne_ch:
                            nc.sync.dma_start(out=wt[ci][:], in_=w_d[(e, ci)].ap())
                    if mt[e] > 2:
                        _load_any(2, nc.scalar)
                    for m in range(3, mt[e]):
                        _load_any(m, nc.sync if m % 2 else nc.scalar)
                    for ci in range(4, ne_ch):
                        nc.sync.dma_start(out=wt[ci][:], in_=w_d[(e, ci)].ap())

                    def _wt_of(ci, k):
                        if ci == 0:
                            return (wa0, k) if k < KH else (wb0, k - KH)
                        return wt[ci], k

                    for ci, m in _lead_pair_order(ne_ch, mt[e]):
                        _pair(ci, m, _wt_of)
                else:
                    # Non-lead xt loads ride the sync ring with the W chunks:
                    # keeping the scalar queue pure-y-writes means the cast
                    # st-slot gates (which count scalar-queue completions)
                    # can never be held hostage by an xt trigger that waits
                    # on far-future PE progress (a ~6us/pair crawl when the
                    # scheduler interleaves them badly).
                    for m in range(mt[e]):
                        nc.sync.dma_start(out=xts[m][:], in_=xt_d[e].ap()[m])
                    for ci in range(len(widths)):
                        wq = wq_pool.tile(
                            [P, KO, widths[ci]], f16, tag="wq", name="wq"
                        )
                        nc.sync.dma_start(out=wq[:], in_=w_d[(e, ci)].ap())

                        def _wt_of(c, k, _wq=wq):
                            return _wq, k

                        for m in range(mt[e]):
                            _pair(ci, m, _wt_of)
                mi0 += mt[e]
    nc.compile()
    return nc


def _prepare(x, weight, counts):
    """Host-side all-to-all: per-core padded token blocks + weight chunks."""
    starts = np.zeros(G + 1, np.int64)
    np.cumsum(counts, out=starts[1:])
    cnt = counts.reshape(M_CORES, EPC)
    mt = tuple(int(v) for v in np.ceil(cnt / P).astype(np.int64).max(axis=0))

    order = _slot_order(mt)
    ne = len(order)
    in_maps, metas = [], []
    for c in range(M_CORES):
        im = {}
        meta = []
        mi0 = 0
        mi0_by_slot = {}
        for j in order:
            mi0_by_slot[j] = mi0
            mi0 += mt[j]
        for ji, j in enumerate(order):
            g = c * EPC + j
            s, n = int(starts[g]), int(counts[g])
            n = min(n, N_TOK - s) if s < N_TOK else 0
            te = P * mt[j]
            xe = np.zeros((te, D_IN), np.float16)
            if n > 0:
                xe[:n] = x[s : s + n]
            # [te, D_IN] -> [D_IN, te] -> [KO, P, mt, 128] -> [mt, P, KO, 128]
            im[f"xT{j}"] = np.ascontiguousarray(
                xe.T.reshape(KO, P, mt[j], P).transpose(2, 1, 0, 3)
            )
            # weight [D_IN, D_OUT] -> chunks [P, KO, w]
            wg = weight[g].reshape(KO, P, D_OUT).transpose(1, 0, 2).astype(np.float16)
            col = 0
            for ci, wd in enumerate(_widths(ji, ne)):
                im[f"W{j}_{ci}"] = np.ascontiguousarray(wg[:, :, col : col + wd])
                col += wd
            meta.append((mi0_by_slot[j], s, n))
        in_maps.append(im)
        metas.append(meta)
    return mt, in_maps, metas


def _ensure_axon_hooks_shim():
    """bass_utils imports antenv.axon_hooks when tracing is requested (e.g.
    via a BASS_TRACE env var); some images lack that module. Install a no-op
    shim so the run degrades to untraced instead of crashing."""
    try:
        from antenv.axon_hooks import get_axon_ntff_profile_hook  # noqa: F401
        return
    except ImportError:
        pass
    import sys
    import types

    try:
        import antenv
    except ImportError:
        return
    mod = types.ModuleType("antenv.axon_hooks")
    mod._hook = None
    mod.get_axon_ntff_profile_hook = lambda: getattr(mod, "_hook", None)

    def _set(h):
        mod._hook = h

    mod.set_axon_ntff_profile_hook = _set
    sys.modules["antenv.axon_hooks"] = mod
    antenv.axon_hooks = mod


def _run(x, weight, counts, trace=False, trace_cores=None):
    from concourse.bass_utils import run_bass_kernel_spmd

    _ensure_axon_hooks_shim()

    x = np.ascontiguousarray(np.asarray(x, dtype=np.float32))
    weight = np.ascontiguousarray(np.asarray(weight, dtype=np.float32))
    counts = np.asarray(counts).astype(np.int64)
    assert counts.shape == (G,)

    mt, in_maps, metas = _prepare(x, weight, counts)
    if sum(mt) == 0:
        return np.zeros((N_TOK, D_OUT), np.float32), None
    if mt not in _cache:
        _cache[mt] = _build(mt)
    nc = _cache[mt]

    res = run_bass_kernel_spmd(
        nc,
        in_maps,
        core_ids=list(range(M_CORES)),
        trace=trace,
        trace_cores=trace_cores,
    )
    out = np.zeros((N_TOK, D_OUT), np.float32)
    for c in range(M_CORES):
        yc = res.results[c]["y"]  # [n_mtiles, P, D_OUT] fp16
        n_mtiles = yc.shape[0]
        yc = yc.reshape(n_mtiles * P, D_OUT).astype(np.float32)
        for mi0, s, n in metas[c]:
            if n > 0:
                out[s : s + n] = yc[mi0 * P : mi0 * P + n]
    return out, res


def kernel(x, weight, num_inputs_per_group):
    out, _ = _run(x, weight, num_inputs_per_group)
    return out

